# revision 1
# baseline (speedup 1.0000x reference)
"""Distributed kNN retrieval kernel for Trainium2 (8 NeuronCores).

Computes: ||x - y|| / 2 + mean(10 smallest ||data_i - x||)  over 2M rows.

Strategy (per the standard distributed-kNN recipe):
  - Shard `data` row-wise across 8 cores (250k rows each, padded to 251,904).
  - Each core's shard is laid out transposed on host: dataT [D=128, N_c] so the
    feature dim sits on SBUF partitions.  Then:
      ACT:  sq = Square(dataT + (-x))        (bias is per-partition = per-dim)
      PE :  psum[t, :] -= sum_d sq[d, :]     (stationary = -1 basis column,
                                              tile index t = output partition)
      ACT:  v = 4096 - d^2                   (PSUM -> SBUF evacuation)
      DVE:  max8 x2 + match_replace          -> top-16 candidates/partition
  - Host gathers 8 x [128,16] candidate values and reduces to the global
    top-10, then finishes the scalar math in numpy.

The kernel streams 1 MiB tiles; the whole thing is HBM-bandwidth bound
(~125 MB/core) with ACT/PE/DVE all comfortably under the DMA roofline.
"""

import numpy as np

import concourse.bacc as bacc
import concourse.mybir as mybir
from concourse.bass_utils import run_bass_kernel_spmd
from concourse.tile import TileContext

D = 128                 # feature dim
N_DATA = 2_000_000      # total database rows
NB_SOFTMIN = 10
MANIFOLD_SPEED = 2.0
N_CORES = 8

F = 2048                # rows per tile (free dim of one streamed tile)
TILES = 123             # tiles per core
N_C = F * TILES         # padded rows per core = 251,904
ROWS_PER_CORE = N_DATA // N_CORES  # 250,000
C_OFF = 4096.0          # v = C_OFF - d^2  (keeps values positive, low ulp)
PAD_VAL = 100.0         # pad-row fill -> d^2 ~ 1.3e6, never in top-k
NEG_BIG = -3.0e38       # match_replace fill

_CACHE = {}


def _n_c(f):
    return f * ((ROWS_PER_CORE + f - 1) // f)


def _build_nc(reps=1, f=F, mode="full", data_bufs=3, sq_bufs=3,
              dma_mix=False, batch=1, inplace=False):
    tiles = _n_c(f) // f
    chunks = f // 512
    nc = bacc.Bacc("TRN2")
    data_t = nc.dram_tensor("data_t", [D, _n_c(f)], mybir.dt.float32,
                            kind="ExternalInput")
    neg_x = nc.dram_tensor("neg_x", [D, 1], mybir.dt.float32,
                           kind="ExternalInput")
    m2x = nc.dram_tensor("m2x", [D, 1], mybir.dt.float32,
                         kind="ExternalInput")
    bias_v = nc.dram_tensor("bias_v", [D, 1], mybir.dt.float32,
                            kind="ExternalInput")
    bconst = nc.dram_tensor("bconst", [D, 256], mybir.dt.float32,
                            kind="ExternalInput")
    bconst_bf = nc.dram_tensor("bconst_bf", [D, 256], mybir.dt.bfloat16,
                               kind="ExternalInput")
    cand = nc.dram_tensor("cand", [D, 16], mybir.dt.float32,
                          kind="ExternalOutput")

    FT = mybir.dt.float32
    AF = mybir.ActivationFunctionType

    with TileContext(nc) as tc:
        with (
            tc.tile_pool(name="consts", bufs=1) as consts,
            tc.tile_pool(name="data", bufs=data_bufs) as data_pool,
            tc.tile_pool(name="sq", bufs=sq_bufs) as sq_pool,
            tc.tile_pool(name="sq2", bufs=sq_bufs) as sq_pool2,
            tc.tile_pool(name="store", bufs=1) as store,
            tc.tile_pool(name="psum", bufs=1, space="PSUM") as psum_pool,
        ):
            mx_sb = consts.tile([D, 1], FT)
            nc.sync.dma_start(out=mx_sb[:, :], in_=neg_x[:, :])
            m2x_sb = consts.tile([D, 1], FT)
            nc.sync.dma_start(out=m2x_sb[:, :], in_=m2x[:, :])
            bias_sb = consts.tile([D, 1], FT)
            nc.sync.dma_start(out=bias_sb[:, :], in_=bias_v[:, :])
            b_sb = consts.tile([D, 256], FT)
            nc.sync.dma_start(out=b_sb[:, :], in_=bconst[:, :])
            b_sb_bf = consts.tile([D, 256], mybir.dt.bfloat16)
            nc.sync.dma_start(out=b_sb_bf[:, :], in_=bconst_bf[:, :])

            pacc = psum_pool.tile([D, chunks * 512], FT)

            import contextlib
            rep_loop = (tc.For_i(0, reps, 1) if reps > 1
                        else contextlib.nullcontext())
            with rep_loop:
                _body(nc, tc, data_t, cand, mx_sb, m2x_sb, bias_sb, b_sb,
                      b_sb_bf, pacc, data_pool, sq_pool, sq_pool2, store, AF,
                      FT, f, tiles, chunks, mode, dma_mix, batch, inplace)

    nc.compile()
    return nc


def _body(nc, tc, data_t, cand, mx_sb, m2x_sb, bias_sb, b_sb, b_sb_bf, pacc,
          data_pool, sq_pool, sq_pool2, store, AF, FT, f, tiles, chunks,
          mode, dma_mix, batch=1, inplace=False):
    import concourse.mybir as mybir
    BF = mybir.dt.bfloat16
    if True:
        if True:
            for b0 in range(0, tiles, batch):
              bts = range(b0, min(b0 + batch, tiles))
              sqs = {}
              for t in bts:
                if mode == "dma_pe_bf":
                    dt_tile = data_pool.tile([D, f], BF)
                    nc.gpsimd.dma_start(out=dt_tile[:, :],
                                        in_=data_t[:, t * f:(t + 1) * f])
                    sqs[t] = dt_tile
                    continue
                dt_tile = data_pool.tile([D, f], FT)
                eng = nc.scalar if (dma_mix and t % 2) else nc.sync
                eng.dma_start(out=dt_tile[:, :],
                              in_=data_t[:, t * f:(t + 1) * f])
                if mode == "dma":
                    continue
                if mode.startswith("dma_pe"):
                    sqs[t] = dt_tile
                    continue
                if mode == "bf":
                    sq = sq_pool.tile([D, f], BF)
                    nc.scalar.activation(out=sq[:, :], in_=dt_tile[:, :],
                                         func=AF.Square, bias=mx_sb[:, :],
                                         scale=1.0)
                    sqs[t] = sq
                    continue
                use_dve = (mode == "dve") or (mode == "split" and t % 2 == 1)
                if inplace:
                    sq = dt_tile
                else:
                    sq = (sq_pool2 if (mode == "split" and use_dve)
                          else sq_pool).tile([D, f], FT)
                if use_dve:
                    # sq = (a - 2x_d) * a = a^2 - 2 x_d a  (sums to d^2-|x|^2)
                    nc.vector.scalar_tensor_tensor(
                        out=sq[:, :], in0=dt_tile[:, :], scalar=m2x_sb[:, :],
                        in1=dt_tile[:, :], op0=mybir.AluOpType.add,
                        op1=mybir.AluOpType.mult)
                else:
                    nc.scalar.activation(out=sq[:, :], in_=dt_tile[:, :],
                                         func=AF.Square, bias=mx_sb[:, :],
                                         scale=1.0)
                sqs[t] = sq
              if mode == "dma" or mode == "dma_act":
                  continue
              nj = 2 if mode == "dma_pe2" else chunks
              use_bf = mode in ("bf", "dma_pe_bf")
              for t in bts:
                for j in range(nj):
                    if mode == "dma_pe_fixw":
                        lhsT = b_sb[:, 0:128]
                    elif use_bf:
                        lhsT = b_sb_bf[:, 128 - t:256 - t]
                    else:
                        lhsT = b_sb[:, 128 - t:256 - t]
                    nc.tensor.matmul(
                        pacc[:, j * 512:(j + 1) * 512],
                        lhsT,
                        sqs[t][:, j * 512:(j + 1) * 512],
                        start=(t == 0),
                        stop=(t == tiles - 1),
                    )

            if (mode in ("full", "dve", "split", "bf")
                    or mode.startswith("dma_pe")):
                # v = C_OFF - d^2 (rows of pacc hold -d^2 per 512-row chunk)
                v = store.tile([D, chunks * 512], FT)
                for j in range(chunks):
                    nc.scalar.activation(out=v[:, j * 512:(j + 1) * 512],
                                         in_=pacc[:, j * 512:(j + 1) * 512],
                                         func=AF.Identity,
                                         bias=bias_sb[:, :], scale=1.0)

                # Top-16 values per partition: max8, zap them, max8 again.
                t8a = store.tile([D, 8], FT)
                nc.vector.max(out=t8a[:, :], in_=v[:, :])
                vrep = store.tile([D, chunks * 512], FT)
                nc.vector.match_replace(out=vrep[:, :],
                                        in_to_replace=t8a[:, :],
                                        in_values=v[:, :],
                                        imm_value=NEG_BIG)
                t8b = store.tile([D, 8], FT)
                nc.vector.max(out=t8b[:, :], in_=vrep[:, :])
            else:
                # Diagnostic modes: emit a token result so the NEFF has
                # a data-dependent output.
                t8a = store.tile([D, 8], FT)
                t8b = store.tile([D, 8], FT)
                src_t = dt_tile if mode in ("dma", "dma_pe") else sq
                nc.vector.max(out=t8a[:, :], in_=src_t[:, 0:512])
                nc.vector.max(out=t8b[:, :], in_=src_t[:, 0:512])

            nc.sync.dma_start(out=cand[:, 0:8], in_=t8a[:, :])
            nc.sync.dma_start(out=cand[:, 8:16], in_=t8b[:, :])


def _get_nc():
    if "nc" not in _CACHE:
        _CACHE["nc"] = _build_nc()
    return _CACHE["nc"]


def _make_in_maps(x, data, f=F, mode="full"):
    n_c = _n_c(f)
    tiles = n_c // f
    neg_x = np.ascontiguousarray((-x).reshape(D, 1), dtype=np.float32)
    m2x = np.ascontiguousarray((-2.0 * x).reshape(D, 1), dtype=np.float32)
    xsq = np.float32(np.dot(x.astype(np.float32), x.astype(np.float32)))
    # Evacuation bias per psum partition (= tile index): v = bias + psum.
    # ACT-path tiles: psum = -d^2          -> bias = C_OFF
    # DVE-path tiles: psum = -d^2 + |x|^2  -> bias = C_OFF - |x|^2
    bias_v = np.full((D, 1), C_OFF, dtype=np.float32)
    if mode == "dve":
        bias_v[:, :] = C_OFF - xsq
    elif mode == "split":
        for t in range(min(tiles, D)):
            if t % 2 == 1:
                bias_v[t, 0] = C_OFF - xsq
    # PSUM partitions with no tile mapped to them (t >= tiles) evacuate as
    # v = bias + 0; poison them so they can never enter the top-k.
    bias_v[tiles:, :] = -1.0e30
    bconst = np.zeros((D, 256), dtype=np.float32)
    bconst[:, 128] = -1.0
    import ml_dtypes
    bconst_bf = bconst.astype(ml_dtypes.bfloat16)
    in_maps = []
    for c in range(N_CORES):
        lo = c * ROWS_PER_CORE
        hi = lo + ROWS_PER_CORE
        shard_t = np.full((D, n_c), PAD_VAL, dtype=np.float32)
        shard_t[:, :ROWS_PER_CORE] = data[lo:hi].T
        in_maps.append({
            "data_t": np.ascontiguousarray(shard_t),
            "neg_x": neg_x,
            "m2x": m2x,
            "bias_v": bias_v,
            "bconst": bconst,
            "bconst_bf": bconst_bf,
        })
    return in_maps


def _postprocess(x, y, results):
    cands = np.concatenate(
        [np.asarray(r["cand"], dtype=np.float32).reshape(-1) for r in results]
    )
    d2 = C_OFF - cands
    # Untouched PSUM rows (tile partitions 123-127) evacuate as exactly
    # C_OFF -> d2 == 0.  Real distances are strictly positive; drop them.
    d2 = d2[d2 > 1e-6]
    d2.sort()
    closest = np.sqrt(d2[:NB_SOFTMIN].astype(np.float32))
    xy = np.float32(np.linalg.norm((x - y).astype(np.float32)))
    return np.float32(xy / np.float32(MANIFOLD_SPEED)
                      + closest.mean(dtype=np.float32))


def kernel(x, y, data, _trace=False):
    x = np.asarray(x, dtype=np.float32)
    y = np.asarray(y, dtype=np.float32)
    data = np.asarray(data, dtype=np.float32)
    nc = _get_nc()
    in_maps = _make_in_maps(x, data)
    res = run_bass_kernel_spmd(nc, in_maps, core_ids=list(range(N_CORES)),
                               trace=_trace)
    out = _postprocess(x, y, res.results)
    if _trace:
        return out, res
    return out



# revision 3
# speedup vs baseline: 2.1713x; 2.1713x over previous
"""Distributed kNN retrieval kernel for Trainium2 (8 NeuronCores).

Computes: ||x - y|| / 2 + mean(10 smallest ||data_i - x||)  over 2M rows.

Strategy (v2 — fp8 streaming):
  - Shard `data` row-wise across 8 cores (250k rows each, padded to 253,952).
  - Host converts each shard to fp8 E3M4 (4-bit mantissa; data ~N(0,1) so
    quantization error ~0.2%, final rel-err ~3e-4, gate is 2e-2) and lays it
    out transposed: data8 [D=128, N_c] so the feature dim sits on SBUF
    partitions and tiles stream 4 KiB/partition.
  - Per tile t (F=4096 rows), one of three engines squares it into fp8 E4M3:
      ACT : sq = Square(a + (-x))    -> psum_t = -d^2
      DVE : sq = (a + (-2x)) * a     -> psum_t = -(d^2 - |x|^2)
      POOL: sq = a * a               -> psum_t = -sum(a^2) [+ 2<a,x> via a
            second matmul over the raw tile with weights 2x] = -(d^2 - |x|^2)
  - PE reduces over dims with DoubleRow fp8 matmuls (2 k-tiles = the two
    paired sq tiles; shifted -1 basis maps tile index -> psum partition),
    accumulated across all 31 pairs into one PSUM [128, 4096] f32 region.
  - DVE max8 on each half of PSUM -> top-8 values per (partition, half)
    = 16 candidates/partition, DMA'd out raw.  Host undoes the per-tile
    engine bias (+|x|^2 for DVE/POOL tiles), reduces 8 cores x 62 x 16
    candidates to the global top-10 and finishes the scalar math.

Roofline: per core 31 MiB of fp8 @ ~330 GB/s ~ 98 us DMA; squares split
ACT/DVE/GPSIMD ~ 95 us; PE DoubleRow + pool cross-terms ~ 50-75 us.
"""

import numpy as np
import ml_dtypes

import concourse.bacc as bacc
import concourse.mybir as mybir
from concourse.bass_utils import run_bass_kernel_spmd
from concourse.tile import TileContext

D = 128                 # feature dim
N_DATA = 2_000_000      # total database rows
NB_SOFTMIN = 10
MANIFOLD_SPEED = 2.0
N_CORES = 8

F = 4096                # rows per tile
ROWS_PER_CORE = N_DATA // N_CORES  # 250,000
TILES = (ROWS_PER_CORE + F - 1) // F   # 62
N_C = F * TILES         # padded rows per core = 253,952
PAIRS = TILES // 2      # 31
PAD_VAL = 8.0           # pad-row fill -> d^2 ~ 8e3, never in top-k

# Default engine schedule counts (ACT, DVE, POOL) summing to TILES.
SCHED = (28, 22, 12)

_CACHE = {}


def _schedule(na=SCHED[0], nd=SCHED[1], npl=SCHED[2]):
    """Greedy interleave of engine assignments by projected finish time."""
    assert na + nd + npl == TILES
    per = {"A": 3.414, "D": 4.267, "P": 7.6}   # us per tile
    left = {"A": na, "D": nd, "P": npl}
    busy = {"A": 0.0, "D": 0.0, "P": 0.0}
    out = []
    for _ in range(TILES):
        cand = [e for e in ("A", "D", "P") if left[e]]
        e = min(cand, key=lambda e: busy[e] + per[e])
        out.append(e)
        left[e] -= 1
        busy[e] += per[e]
    return "".join(out)


def _build_nc(sched=None, dma_mix=False):
    sched = sched or _schedule()
    assert len(sched) == TILES
    nc = bacc.Bacc("TRN2")
    data8 = nc.dram_tensor("data8", [D, N_C], mybir.dt.float8e3,
                           kind="ExternalInput")
    neg_x = nc.dram_tensor("neg_x", [D, 1], mybir.dt.float32,
                           kind="ExternalInput")
    m2x = nc.dram_tensor("m2x", [D, 1], mybir.dt.float32,
                         kind="ExternalInput")
    wconst = nc.dram_tensor("wconst", [D, 2, 192], mybir.dt.float8e4,
                            kind="ExternalInput")
    wx2 = nc.dram_tensor("wx2", [D, 256], mybir.dt.float8e3,
                         kind="ExternalInput")
    cand = nc.dram_tensor("cand", [D, 16], mybir.dt.float32,
                          kind="ExternalOutput")

    FT = mybir.dt.float32
    F83 = mybir.dt.float8e3
    F84 = mybir.dt.float8e4
    AF = mybir.ActivationFunctionType
    ALU = mybir.AluOpType
    DR = mybir.MatmulPerfMode.DoubleRow
    CHUNKS = F // 512

    with TileContext(nc) as tc:
        with (
            tc.tile_pool(name="consts", bufs=1) as consts,
            tc.tile_pool(name="data", bufs=4) as data_pool,
            tc.tile_pool(name="sq", bufs=2) as sq_pool,
            tc.tile_pool(name="store", bufs=1) as store,
            tc.tile_pool(name="psum", bufs=1, space="PSUM") as psum_pool,
        ):
            mx_sb = consts.tile([D, 1], FT)
            nc.sync.dma_start(out=mx_sb[:, :], in_=neg_x[:, :])
            m2x_sb = consts.tile([D, 1], FT)
            nc.sync.dma_start(out=m2x_sb[:, :], in_=m2x[:, :])
            wc_sb = consts.tile([D, 2, 192], F84)
            nc.sync.dma_start(out=wc_sb[:, :, :], in_=wconst[:, :, :])
            wx_sb = consts.tile([D, 256], F83)
            nc.sync.dma_start(out=wx_sb[:, :], in_=wx2[:, :])

            pacc = psum_pool.tile([D, F], FT)
            first = [True]

            def mm(out_ap, lhsT, rhs, is_last, **kw):
                nc.tensor.matmul(out_ap, lhsT, rhs,
                                 start=first[0], stop=is_last, **kw)

            for k in range(PAIRS):
                sqp = sq_pool.tile([D, 2, F], F84)
                raw_pool = []           # (tile_index, data tile) for POOL tiles
                for i in (0, 1):
                    t = 2 * k + i
                    dt_tile = data_pool.tile([D, F], F83)
                    eng_q = nc.scalar if (dma_mix and t % 2) else nc.sync
                    eng_q.dma_start(out=dt_tile[:, :],
                                    in_=data8[:, t * F:(t + 1) * F])
                    e = sched[t]
                    if e == "A":
                        nc.scalar.activation(out=sqp[:, i, :],
                                             in_=dt_tile[:, :],
                                             func=AF.Square, bias=mx_sb[:, :],
                                             scale=1.0)
                    elif e == "D":
                        nc.vector.scalar_tensor_tensor(
                            out=sqp[:, i, :], in0=dt_tile[:, :],
                            scalar=m2x_sb[:, :], in1=dt_tile[:, :],
                            op0=ALU.add, op1=ALU.mult)
                    else:
                        nc.gpsimd.tensor_tensor(
                            out=sqp[:, i, :], in0=dt_tile[:, :],
                            in1=dt_tile[:, :], op=ALU.mult)
                        raw_pool.append((t, dt_tile))
                # cross-term matmuls for POOL tiles (weights 2x at col t),
                # grouped per tile so PE weight loads stay batched
                for t, dt_tile in raw_pool:
                    for j in range(CHUNKS):
                        mm(pacc[:, j * 512:(j + 1) * 512],
                           wx_sb[:, 128 - t:256 - t],
                           dt_tile[:, j * 512:(j + 1) * 512],
                           is_last=False)
                        first[0] = False
                for j in range(CHUNKS):
                    mm(pacc[:, j * 512:(j + 1) * 512],
                       wc_sb[:, :, 64 - 2 * k:192 - 2 * k],
                       sqp[:, :, j * 512:(j + 1) * 512],
                       is_last=(k == PAIRS - 1),
                       perf_mode=DR)
                    first[0] = False

            t8a = store.tile([D, 8], FT)
            nc.vector.max(out=t8a[:, :], in_=pacc[:, 0:F // 2])
            t8b = store.tile([D, 8], FT)
            nc.vector.max(out=t8b[:, :], in_=pacc[:, F // 2:F])
            nc.sync.dma_start(out=cand[:, 0:8], in_=t8a[:, :])
            nc.sync.dma_start(out=cand[:, 8:16], in_=t8b[:, :])

    nc.compile()
    return nc, sched


def _get_nc():
    if "nc" not in _CACHE:
        _CACHE["nc"] = _build_nc()
    return _CACHE["nc"]


def _make_in_maps(x, data):
    neg_x = np.ascontiguousarray((-x).reshape(D, 1), dtype=np.float32)
    m2x = np.ascontiguousarray((-2.0 * x).reshape(D, 1), dtype=np.float32)
    wconst = np.zeros((D, 2, 192), dtype=ml_dtypes.float8_e4m3)
    wconst[:, 0, 64] = -1.0
    wconst[:, 1, 65] = -1.0
    wx2 = np.zeros((D, 256), dtype=ml_dtypes.float8_e3m4)
    wx2[:, 128] = (2.0 * x).astype(ml_dtypes.float8_e3m4)

    data8 = data.astype(ml_dtypes.float8_e3m4)          # [N, D]
    in_maps = []
    for c in range(N_CORES):
        lo = c * ROWS_PER_CORE
        shard = np.full((D, N_C), PAD_VAL, dtype=ml_dtypes.float8_e3m4)
        shard[:, :ROWS_PER_CORE] = data8[lo:lo + ROWS_PER_CORE].T
        in_maps.append({
            "data8": np.ascontiguousarray(shard),
            "neg_x": neg_x,
            "m2x": m2x,
            "wconst": wconst,
            "wx2": wx2,
        })
    return in_maps


def _postprocess(x, y, results, sched):
    xsq = np.float32(np.dot(x.astype(np.float32), x.astype(np.float32)))
    d2_all = []
    for r in results:
        c = np.asarray(r["cand"], dtype=np.float32)     # [D, 16] raw psum max
        d2 = -c[:TILES, :]                              # ACT tiles: -psum = d^2
        for t in range(TILES):
            if sched[t] != "A":
                d2[t, :] += xsq                         # DVE/POOL: + |x|^2
        d2_all.append(d2.reshape(-1))
    d2 = np.concatenate(d2_all)
    d2 = d2[d2 > 1e-6]
    d2.sort()
    closest = np.sqrt(d2[:NB_SOFTMIN].astype(np.float32))
    xy = np.float32(np.linalg.norm((x - y).astype(np.float32)))
    return np.float32(xy / np.float32(MANIFOLD_SPEED)
                      + closest.mean(dtype=np.float32))


def kernel(x, y, data, _trace=False):
    x = np.asarray(x, dtype=np.float32)
    y = np.asarray(y, dtype=np.float32)
    data = np.asarray(data, dtype=np.float32)
    nc, sched = _get_nc()
    in_maps = _make_in_maps(x, data)
    res = run_bass_kernel_spmd(nc, in_maps, core_ids=list(range(N_CORES)),
                               trace=_trace)
    out = _postprocess(x, y, res.results, sched)
    if _trace:
        return out, res
    return out
